# revision 23
# baseline (speedup 1.0000x reference)
"""Trainium2 Bass kernel for BinarizeConv2dSDP.

Reference math (forward only):
    w    = rsqrt(m^2 + sum_k z_k^2/100) * (m + rv @ z)   elementwise
    bw   = sign(w)        -- the positive rsqrt factor drops out of sign()
    ba   = sign(x)
    out  = conv2d(ba, bw, pad=1, NCHW/OIHW) * alpha[o]

Sharding (8 cores, no collectives): 2D grid, batch 4-way x out-channel
2-way. Core i handles images [16*(i//2), 16*(i//2)+16) and out-channels
[128*(i%2), 128*(i%2)+128). Outputs are disjoint. (An AllGather-based
variant with 8-way weight sharding was tried and abandoned: the ncfw
collective path costs 30-65 us on first call here, swamping the saved
DMA.)

Per-core pipeline:
  - Z slice lands host-transposed as [128 part(c_low), 8 k, 2304
    (cc t oo)] so the SDP fold is 8 contiguous fused DVE ops (acc =
    rv_k*z_k + acc, M folded into the k=0 op), each chasing its own Z
    k-chunk DMA; rv is partition-broadcast by a single ones-matmul.
    Two ACT signs emit the fp8 conv weights wt [128 c_low, 2 cc, 9 t,
    128 o] directly — no PE transposes, no reshuffles.
  - a chain of dummy DoubleRow matmuls (garbage data, dead psum) is
    released after fold k=5 so the PE's HAM clock-gate is warm when the
    real conv starts ~3 us later.
  - activations: per image [128 part(c_low), 2 c-chunk, 912] fp8 zero-
    padded 30x30 images; border memsets on DVE (dep-free, run during the
    Z window); 2 images binarized on DVE to +-0.5 (drains use 2*alpha),
    14 signed on ACT behind the weight signs.
  - conv: per image, the two half-image psum groups are interleaved
    tap-by-tap so each tap's DoubleRow LDWEIGHTS is shared by two
    matmuls; psum drains (x alpha) on DVE; outputs ride the ACT HWDGE
    ring while inputs keep the SP ring.
"""

import sys

for _p in ("/opt/trn_rl_repo",):
    if _p not in sys.path:
        sys.path.insert(0, _p)

import contextlib

import numpy as np

import concourse.bass as bass
import concourse.bacc as bacc
import concourse.tile as tile
from concourse import mybir
from concourse.bass_utils import run_bass_kernel_spmd
from concourse.tile_rust import add_dep_helper

N_CORES = 8
B = 64
B_SH = 16       # images per core (batch/4)
C = 256         # in channels
O = 256
O_SH = 128      # out channels per core (o/2)
K = 8           # SDP rank
KK = 9          # 3x3 taps
CT = C * KK     # 2304
H = 28
HP = 30         # padded row width
PADW = 912      # 30*30=900 padded to %16
WCOLS = 2 * KK * O_SH  # 2304 weight cols per core (cc t oo)
F32 = mybir.dt.float32
FP8 = mybir.dt.float8e4


def _build_kernel(tc, x_t, z_t, m_t, a_t, rv_t, ones_t, out_t):
    nc = tc.nc
    ctx = contextlib.ExitStack()
    consts = ctx.enter_context(tc.tile_pool(name="consts", bufs=1))
    zpool = ctx.enter_context(tc.tile_pool(name="zpool", bufs=1))
    stage = ctx.enter_context(tc.tile_pool(name="stage", bufs=8))
    acts = ctx.enter_context(tc.tile_pool(name="acts", bufs=1))
    outp = ctx.enter_context(tc.tile_pool(name="outp", bufs=8))
    psums = ctx.enter_context(tc.tile_pool(name="psums", bufs=6, space="PSUM"))
    pst = ctx.enter_context(tc.tile_pool(name="pst", bufs=2, space="PSUM"))

    with ctx:
        # ---- tiny constants on the gpsimd HWDGE ring; rv partition-
        # broadcast via a K=1 ones-matmul on the idle PE ----
        rv_raw = consts.tile([1, K], F32, name="rv_raw")
        nc.gpsimd.dma_start(rv_raw, rv_t.ap())
        ones_sb = consts.tile([1, 128], F32, name="ones_sb")
        nc.gpsimd.dma_start(ones_sb, ones_t.ap())
        alpha_sb = consts.tile([128, 1], F32, name="alpha_sb")
        nc.gpsimd.dma_start(alpha_sb, a_t.ap())
        ps_rv = pst.tile([128, K], F32, name="ps_rv", tag="pst")
        nc.tensor.matmul(ps_rv, ones_sb, rv_raw, start=True, stop=True)
        rv_sb = consts.tile([128, K], F32, name="rv_sb")
        nc.vector.tensor_copy(rv_sb, ps_rv)
        # 2*alpha for the images binarized on DVE to +-0.5 (see below)
        alpha2_sb = consts.tile([128, 1], F32, name="alpha2_sb")
        nc.vector.tensor_scalar_mul(alpha2_sb, alpha_sb, 2.0)

        # ---- weight inputs: M rides the ACT ring (its dma_start stalls
        # the issuing engine ~14us on ring backpressure behind z, and ACT
        # is the only engine idle until the weight signs at ~38us) ----
        m_sb = zpool.tile([128, WCOLS], F32, name="m_sb")
        nc.scalar.dma_start(m_sb, m_t.ap())
        z_sb = zpool.tile([128, K, WCOLS], F32, name="z_sb")
        HW2 = WCOLS // 2
        xst = []
        for n in range(B_SH):
            xst.append(stage.tile([128, 2, H * H], F32, name=f"xst{n}", tag="xst"))

        def x_dma(n):
            xr = x_t.ap()[n].rearrange("(cc p) h w -> p cc (h w)", p=128)
            for cc in range(2):
                nc.sync.dma_start(xst[n][:, cc, :], xr[:, cc, :])

        for k in range(K):
            if k == K - 1:
                # image 0 jumps the queue ahead of the last z chunk so its
                # DVE binarize can overlap the fold/sign tail
                x_dma(0)
            for hh in range(2):
                sl = slice(hh * HW2, (hh + 1) * HW2)
                nc.sync.dma_start(z_sb[:, k, sl], z_t.ap()[:, k, sl])
        # z rides ahead of x in the same SP queue (FIFO), so x can use
        # big 3.1KB-run chunks without stealing the Z window
        for n in range(1, B_SH):
            x_dma(n)

        # ---- act tiles + border memsets up front on the otherwise-idle
        # GpSimd engine: DVE must stay dedicated to the fold (its 16 ops
        # barely track the z stream; any extra DVE work is critical-path)
        act_tiles = []
        for n in range(B_SH):
            a_n = acts.tile([128, 2, PADW], FP8, name=f"a{n}", tag=f"a{n}")
            nc.gpsimd.memset(a_n[:, :, 0:30], 0.0)
            nc.gpsimd.memset(a_n[:, :, 870:PADW], 0.0)
            pairs = a_n[:, :, 29 : 29 + 29 * HP].rearrange(
                "p cc (r two) -> p cc r two", two=HP
            )[:, :, :, :2]
            nc.gpsimd.memset(pairs, 0.0)
            act_tiles.append(a_n)

        # ---- SDP fold on DVE: acc = M + sum_k rv_k * z_k, each k-step
        # chasing its own Z chunk; layout [c_low, cc, t, oo] throughout ----
        # (the Pool engine rejects pointer-scalar ops, so the whole fold
        # stays on DVE — but DVE does nothing else until the drains)
        acc = zpool.tile([128, WCOLS], F32, name="acc")
        fold_ops = []
        for k in range(K):
            for hh in range(2):
                sl = slice(hh * HW2, (hh + 1) * HW2)
                op = nc.vector.scalar_tensor_tensor(
                    acc[:, sl],
                    z_sb[:, k, sl],
                    rv_sb[:, k : k + 1],
                    m_sb[:, sl] if k == 0 else acc[:, sl],
                    op0=mybir.AluOpType.mult,
                    op1=mybir.AluOpType.add,
                )
                fold_ops.append(op)
        # fp8 conv weights, signed straight into conv layout
        wt = consts.tile([128, 2, KK, O_SH], FP8, name="wt")
        wt_flat = wt.rearrange("p cc t o -> p (cc t o)")
        wt_sign = None
        for hh in range(2):
            sl = slice(hh * HW2, (hh + 1) * HW2)
            wt_sign = nc.scalar.sign(wt_flat[:, sl], acc[:, sl])

        # ---- HAM warm-up: chained dummy DoubleRow matmuls on zeros into
        # a dead psum, released late in the fold so the PE clock-gate is
        # at 8/8 when the real conv stream begins ----
        junk = consts.tile([128, 2, 544], FP8, name="junk")
        nc.vector.memset(junk, 0.0)
        ps_warm = pst.tile([128, 420], F32, name="ps_warm", tag="pst")
        warm = None
        for i in range(16):
            warm = nc.tensor.matmul(
                ps_warm,
                junk[:, :, 0:128],
                junk[:, :, 112:532],
                start=True,
                stop=True,
                perf_mode=mybir.MatmulPerfMode.DoubleRow,
            )
            if i == 0:
                add_dep_helper(warm.ins, fold_ops[-3].ins, reason="warmup late in fold")

        # ---- activation binarize: sign(x) per image ----
        for n in range(B_SH):
            a_n = act_tiles[n]
            interior = a_n[:, :, 31 : 31 + 28 * HP].rearrange(
                "p cc (r xx) -> p cc r xx", xx=HP
            )[:, :, :, :28]
            xv = xst[n].rearrange("p cc (h w) -> p cc h w", w=28)
            if n < 2:
                # first two images binarize on GpSimd to (x>=0)-0.5 =
                # +-0.5 — exact in fp8; their drains use 2*alpha — so
                # neither the ACT sign queue nor the DVE fold is touched
                nc.gpsimd.tensor_scalar(
                    interior,
                    xv,
                    0.0,
                    0.5,
                    op0=mybir.AluOpType.is_ge,
                    op1=mybir.AluOpType.subtract,
                )
            else:
                si = nc.scalar.sign(interior, xv)
                add_dep_helper(si.ins, wt_sign.ins, reason="wt signs gate x signs")

        # ---- conv: 9 taps x 2 half-images per image; both halves share
        # each tap's LDWEIGHTS (pair the matmuls) so weight loads hide ----
        for n in range(B_SH):
            a_n = act_tiles[n]
            ps0 = psums.tile([128, 420], F32, name="ps0", tag="ps")
            ps1 = psums.tile([128, 420], F32, name="ps1", tag="ps")
            pss = (ps0, ps1)
            for t in range(KK):
                dy, dx = divmod(t, 3)
                for half in range(2):
                    off = (half * 14 + dy) * HP + dx
                    nc.tensor.matmul(
                        pss[half],
                        wt[:, :, t, :],
                        a_n[:, :, off : off + 420],
                        start=(t == 0),
                        stop=(t == KK - 1),
                        perf_mode=mybir.MatmulPerfMode.DoubleRow,
                    )
            a_col = alpha2_sb if n < 2 else alpha_sb
            for half in range(2):
                ob = outp.tile([128, 392], F32, name="ob", tag="ob")
                ps_v = pss[half].rearrange("p (r xx) -> p r xx", xx=HP)[:, :, :28]
                ob_v = ob.rearrange("p (r xx) -> p r xx", xx=28)
                # drains on DVE so the in-order ACT sign stream can't
                # stall the psum recycle
                nc.vector.tensor_scalar_mul(ob_v, ps_v, a_col[:, 0:1])
                dst = out_t.ap()[n].rearrange("o h w -> o (h w)")[
                    :, half * 392 : (half + 1) * 392
                ]
                # early images' out-writes ride the ACT HWDGE ring (SP is
                # still streaming x); later images split ACT/SP so the
                # tail drains two rings wide
                eng = nc.sync if (n >= 8 and half == 1) else nc.scalar
                eng.dma_start(dst, ob)


_PROGRAM = None


def build_program():
    global _PROGRAM
    if _PROGRAM is not None:
        return _PROGRAM
    nc = bacc.Bacc(
        "TRN2",
        target_bir_lowering=False,
        debug=False,
        enable_asserts=False,
        num_devices=N_CORES,
    )
    x_t = nc.dram_tensor("x", [B_SH, C, H, H], F32, kind="ExternalInput")
    z_t = nc.dram_tensor("Zq", [128, K, WCOLS], F32, kind="ExternalInput")
    m_t = nc.dram_tensor("Mq", [128, WCOLS], F32, kind="ExternalInput")
    a_t = nc.dram_tensor("alphap", [O_SH, 1], F32, kind="ExternalInput")
    rv_t = nc.dram_tensor("rv", [1, K], F32, kind="ExternalInput")
    ones_t = nc.inline_tensor(np.ones((1, 128), dtype=np.float32), name="ones128")
    out_t = nc.dram_tensor("out", [B_SH, O_SH, H, H], F32, kind="ExternalOutput")

    with tile.TileContext(nc) as tc:
        _build_kernel(tc, x_t, z_t, m_t, a_t, rv_t, ones_t, out_t)
    nc.compile()
    _PROGRAM = nc
    return nc


def make_in_maps(x, M, Z, alpha, rv):
    x = np.ascontiguousarray(np.asarray(x, dtype=np.float32))
    M = np.asarray(M, dtype=np.float32).reshape(O, CT)
    Z = np.asarray(Z, dtype=np.float32).reshape(K, O, CT)
    alpha = np.asarray(alpha, dtype=np.float32).reshape(O)
    rv = np.ascontiguousarray(np.asarray(rv, dtype=np.float32))
    in_maps = []
    for i in range(N_CORES):
        b, oh = divmod(i, 2)
        osl = slice(oh * O_SH, (oh + 1) * O_SH)
        # Zq[c_low, k, (cc t oo)] = Z[k, 128*oh+oo, (cc*128+c_low)*9 + t]
        zq = np.ascontiguousarray(
            Z[:, osl].reshape(K, O_SH, 2, 128, KK)
            .transpose(3, 0, 2, 4, 1)
            .reshape(128, K, WCOLS)
        )
        mq = np.ascontiguousarray(
            M[osl].reshape(O_SH, 2, 128, KK).transpose(2, 1, 3, 0).reshape(128, WCOLS)
        )
        in_maps.append(
            {
                "x": np.ascontiguousarray(x[b * B_SH : (b + 1) * B_SH]),
                "Zq": zq,
                "Mq": mq,
                "alphap": np.ascontiguousarray(alpha[osl].reshape(O_SH, 1)),
                "rv": rv,
            }
        )
    return in_maps


def assemble_out(results):
    out = np.empty((B, O, H, H), dtype=np.float32)
    for i in range(N_CORES):
        b, oh = divmod(i, 2)
        r = np.asarray(results[i]["out"]).reshape(B_SH, O_SH, H, H)
        out[b * B_SH : (b + 1) * B_SH, oh * O_SH : (oh + 1) * O_SH] = r
    return out


def kernel(x, M, Z, alpha, rv, trace=False):
    nc = build_program()
    in_maps = make_in_maps(x, M, Z, alpha, rv)
    res = run_bass_kernel_spmd(
        nc, in_maps, core_ids=list(range(N_CORES)), trace=trace
    )
    if trace:
        kernel.last_results = res
    return assemble_out(res.results)


if __name__ == "__main__":
    build_program()
    print("program built ok")


# revision 29
# speedup vs baseline: 1.3564x; 1.3564x over previous
"""Trainium2 Bass kernel for BinarizeConv2dSDP.

Reference math (forward only):
    w    = rsqrt(m^2 + sum_k z_k^2/100) * (m + rv @ z)   elementwise
    bw   = sign(w)        -- the positive rsqrt factor drops out of sign()
    ba   = sign(x)
    out  = conv2d(ba, bw, pad=1, NCHW/OIHW) * alpha[o]

Sharding (8 cores, no collectives): 2D grid, batch 4-way x out-channel
2-way. Core i handles images [16*(i//2), 16*(i//2)+16) and out-channels
[128*(i%2), 128*(i%2)+128). Outputs are disjoint. (An AllGather-based
variant with 8-way weight sharding was tried and abandoned: the ncfw
collective path costs 30-65 us on first call here, swamping the saved
DMA.)

Per-core pipeline:
  - Z slice lands host-transposed as [128 part(c_low), 8 k, 2304
    (cc t oo)] so the SDP fold is 8 contiguous fused DVE ops (acc =
    rv_k*z_k + acc, M folded into the k=0 op), each chasing its own Z
    k-chunk DMA; rv is partition-broadcast by a single ones-matmul.
    Two ACT signs emit the fp8 conv weights wt [128 c_low, 2 cc, 9 t,
    128 o] directly — no PE transposes, no reshuffles.
  - a chain of dummy DoubleRow matmuls (garbage data, dead psum) is
    released after fold k=5 so the PE's HAM clock-gate is warm when the
    real conv starts ~3 us later.
  - activations: per image [128 part(c_low), 2 c-chunk, 912] fp8 zero-
    padded 30x30 images; border memsets on DVE (dep-free, run during the
    Z window); 2 images binarized on DVE to +-0.5 (drains use 2*alpha),
    14 signed on ACT behind the weight signs.
  - conv: per image, the two half-image psum groups are interleaved
    tap-by-tap so each tap's DoubleRow LDWEIGHTS is shared by two
    matmuls; psum drains (x alpha) on DVE; outputs ride the ACT HWDGE
    ring while inputs keep the SP ring.
"""

import sys

for _p in ("/opt/trn_rl_repo",):
    if _p not in sys.path:
        sys.path.insert(0, _p)

import contextlib

import numpy as np

import concourse.bass as bass
import concourse.bacc as bacc
import concourse.tile as tile
from concourse import mybir
from concourse.bass_utils import run_bass_kernel_spmd
from concourse.tile_rust import add_dep_helper

N_CORES = 8
B = 64
B_SH = 16       # images per core (batch/4)
C = 256         # in channels
O = 256
O_SH = 128      # out channels per core (o/2)
K = 8           # SDP rank
KK = 9          # 3x3 taps
CT = C * KK     # 2304
H = 28
HP = 30         # padded row width
PADW = 912      # 30*30=900 padded to %16
WCOLS = 2 * KK * O_SH  # 2304 weight cols per core (cc t oo)
F32 = mybir.dt.float32
FP8 = mybir.dt.float8e4


def _build_kernel(tc, x_t, z_t, m_t, a_t, rv_t, ones_t, out_t):
    nc = tc.nc
    ctx = contextlib.ExitStack()
    consts = ctx.enter_context(tc.tile_pool(name="consts", bufs=1))
    zpool = ctx.enter_context(tc.tile_pool(name="zpool", bufs=1))
    stage = ctx.enter_context(tc.tile_pool(name="stage", bufs=8))
    acts = ctx.enter_context(tc.tile_pool(name="acts", bufs=1))
    outp = ctx.enter_context(tc.tile_pool(name="outp", bufs=8))
    psums = ctx.enter_context(tc.tile_pool(name="psums", bufs=6, space="PSUM"))
    pst = ctx.enter_context(tc.tile_pool(name="pst", bufs=2, space="PSUM"))

    with ctx:
        # ---- tiny constants on the gpsimd HWDGE ring; rv partition-
        # broadcast via a K=1 ones-matmul on the idle PE ----
        rv_raw = consts.tile([1, K], F32, name="rv_raw")
        nc.gpsimd.dma_start(rv_raw, rv_t.ap())
        ones_sb = consts.tile([1, 128], F32, name="ones_sb")
        nc.gpsimd.dma_start(ones_sb, ones_t.ap())
        alpha_sb = consts.tile([128, 1], F32, name="alpha_sb")
        nc.gpsimd.dma_start(alpha_sb, a_t.ap())
        ps_rv = pst.tile([128, K], F32, name="ps_rv", tag="pst")
        nc.tensor.matmul(ps_rv, ones_sb, rv_raw, start=True, stop=True)
        rv_sb = consts.tile([128, K], F32, name="rv_sb")
        nc.vector.tensor_copy(rv_sb, ps_rv)
        # 2*alpha for the images binarized on DVE to +-0.5 (see below)
        alpha2_sb = consts.tile([128, 1], F32, name="alpha2_sb")
        nc.vector.tensor_scalar_mul(alpha2_sb, alpha_sb, 2.0)

        # ---- weight inputs: M on the (otherwise idle) gpsimd ring; its
        # dma_start stalls that engine on ring backpressure behind z,
        # which is harmless there ----
        m_sb = zpool.tile([128, WCOLS], F32, name="m_sb")
        nc.gpsimd.dma_start(m_sb, m_t.ap())
        z_sb = zpool.tile([128, K, WCOLS], F32, name="z_sb")
        HW2 = WCOLS // 2
        # x arrives host-pre-padded as 30x30(+12) zero-bordered images, so
        # sign(0)=0 builds the conv padding for free on the ACT images
        xst = []
        for n in range(B_SH):
            xst.append(stage.tile([128, 2, PADW], F32, name=f"xst{n}", tag="xst"))

        def x_dma(n):
            xr = x_t.ap()[n].rearrange("(cc p) w -> p cc w", p=128)
            for cc in range(2):
                nc.sync.dma_start(xst[n][:, cc, :], xr[:, cc, :])

        for k in range(K):
            if k == K - 1:
                # image 0 jumps the queue ahead of the last z chunk so its
                # DVE binarize can overlap the fold/sign tail
                x_dma(0)
            for hh in range(2):
                sl = slice(hh * HW2, (hh + 1) * HW2)
                nc.sync.dma_start(z_sb[:, k, sl], z_t.ap()[:, k, sl])
        # z rides ahead of x in the same SP queue (FIFO), so x can use
        # big 3.1KB-run chunks without stealing the Z window
        for n in range(1, B_SH):
            x_dma(n)

        # ---- act tiles; only the two DVE-binarized images need border
        # memsets (dep-free, early) — ACT-signed images get their zero
        # padding from the pre-padded x via sign(0)=0 ----
        act_tiles = []
        for n in range(B_SH):
            a_n = acts.tile([128, 2, PADW], FP8, name=f"a{n}", tag=f"a{n}")
            if n < 2:
                nc.vector.memset(a_n[:, :, 0:30], 0.0)
                nc.vector.memset(a_n[:, :, 870:PADW], 0.0)
                pairs = a_n[:, :, 29 : 29 + 29 * HP].rearrange(
                    "p cc (r two) -> p cc r two", two=HP
                )[:, :, :, :2]
                nc.vector.memset(pairs, 0.0)
            act_tiles.append(a_n)

        # ---- SDP fold on DVE: acc = M + sum_k rv_k * z_k, each k-step
        # chasing its own Z chunk; layout [c_low, cc, t, oo] throughout ----
        # (the Pool engine rejects pointer-scalar ops, so the whole fold
        # stays on DVE — but DVE does nothing else until the drains)
        acc = zpool.tile([128, WCOLS], F32, name="acc")
        fold_ops = []
        for k in range(K):
            for hh in range(2):
                sl = slice(hh * HW2, (hh + 1) * HW2)
                op = nc.vector.scalar_tensor_tensor(
                    acc[:, sl],
                    z_sb[:, k, sl],
                    rv_sb[:, k : k + 1],
                    m_sb[:, sl] if k == 0 else acc[:, sl],
                    op0=mybir.AluOpType.mult,
                    op1=mybir.AluOpType.add,
                )
                fold_ops.append(op)
        # fp8 conv weights, signed straight into conv layout
        wt = consts.tile([128, 2, KK, O_SH], FP8, name="wt")
        wt_flat = wt.rearrange("p cc t o -> p (cc t o)")
        wt_sign = None
        for hh in range(2):
            sl = slice(hh * HW2, (hh + 1) * HW2)
            wt_sign = nc.scalar.sign(wt_flat[:, sl], acc[:, sl])

        # ---- HAM warm-up: chained dummy DoubleRow matmuls on zeros into
        # a dead psum, released late in the fold so the PE clock-gate is
        # at 8/8 when the real conv stream begins ----
        junk = consts.tile([128, 2, 544], FP8, name="junk")
        nc.vector.memset(junk, 0.0)
        ps_warm = pst.tile([128, 420], F32, name="ps_warm", tag="pst")
        warm = None
        for i in range(16):
            warm = nc.tensor.matmul(
                ps_warm,
                junk[:, :, 0:128],
                junk[:, :, 112:532],
                start=True,
                stop=True,
                perf_mode=mybir.MatmulPerfMode.DoubleRow,
            )
            if i == 0:
                add_dep_helper(warm.ins, fold_ops[-3].ins, reason="warmup late in fold")

        # ---- activation binarize: sign(x) per image ----
        for n in range(B_SH):
            a_n = act_tiles[n]
            if n < 2:
                # first two images binarize on DVE (right after the fold)
                # to (x>=0)-0.5 = +-0.5 — exact in fp8; their drains use
                # 2*alpha — keeping the in-order ACT queue free for the
                # critical weight signs; only the 28x28 interior, since
                # (0>=0)-0.5 would corrupt the zero borders
                interior = a_n[:, :, 31 : 31 + 28 * HP].rearrange(
                    "p cc (r xx) -> p cc r xx", xx=HP
                )[:, :, :, :28]
                xv = xst[n][:, :, 31 : 31 + 28 * HP].rearrange(
                    "p cc (r xx) -> p cc r xx", xx=HP
                )[:, :, :, :28]
                nc.vector.tensor_scalar(
                    interior,
                    xv,
                    0.0,
                    0.5,
                    op0=mybir.AluOpType.is_ge,
                    op1=mybir.AluOpType.subtract,
                )
            else:
                # full contiguous tile: the pre-padded zeros sign to 0
                si = nc.scalar.sign(a_n, xst[n])
                add_dep_helper(si.ins, wt_sign.ins, reason="wt signs gate x signs")

        # ---- conv: 9 taps x 2 half-images per image; both halves share
        # each tap's LDWEIGHTS (pair the matmuls) so weight loads hide ----
        for n in range(B_SH):
            a_n = act_tiles[n]
            ps0 = psums.tile([128, 420], F32, name="ps0", tag="ps")
            ps1 = psums.tile([128, 420], F32, name="ps1", tag="ps")
            pss = (ps0, ps1)
            for t in range(KK):
                dy, dx = divmod(t, 3)
                for half in range(2):
                    off = (half * 14 + dy) * HP + dx
                    nc.tensor.matmul(
                        pss[half],
                        wt[:, :, t, :],
                        a_n[:, :, off : off + 420],
                        start=(t == 0),
                        stop=(t == KK - 1),
                        perf_mode=mybir.MatmulPerfMode.DoubleRow,
                    )
            a_col = alpha2_sb if n < 2 else alpha_sb
            for half in range(2):
                ob = outp.tile([128, 392], F32, name="ob", tag="ob")
                ps_v = pss[half].rearrange("p (r xx) -> p r xx", xx=HP)[:, :, :28]
                ob_v = ob.rearrange("p (r xx) -> p r xx", xx=28)
                # drains on DVE so the in-order ACT sign stream can't
                # stall the psum recycle
                nc.vector.tensor_scalar_mul(ob_v, ps_v, a_col[:, 0:1])
                dst = out_t.ap()[n].rearrange("o h w -> o (h w)")[
                    :, half * 392 : (half + 1) * 392
                ]
                # early images' out-writes ride the ACT HWDGE ring (SP is
                # still streaming x); later images split ACT/SP so the
                # tail drains two rings wide
                eng = nc.sync if (n >= 8 and half == 1) else nc.scalar
                eng.dma_start(dst, ob)


_PROGRAM = None


def build_program():
    global _PROGRAM
    if _PROGRAM is not None:
        return _PROGRAM
    nc = bacc.Bacc(
        "TRN2",
        target_bir_lowering=False,
        debug=False,
        enable_asserts=False,
        num_devices=N_CORES,
    )
    x_t = nc.dram_tensor("x", [B_SH, C, PADW], F32, kind="ExternalInput")
    z_t = nc.dram_tensor("Zq", [128, K, WCOLS], F32, kind="ExternalInput")
    m_t = nc.dram_tensor("Mq", [128, WCOLS], F32, kind="ExternalInput")
    a_t = nc.dram_tensor("alphap", [O_SH, 1], F32, kind="ExternalInput")
    rv_t = nc.dram_tensor("rv", [1, K], F32, kind="ExternalInput")
    ones_t = nc.inline_tensor(np.ones((1, 128), dtype=np.float32), name="ones128")
    out_t = nc.dram_tensor("out", [B_SH, O_SH, H, H], F32, kind="ExternalOutput")

    with tile.TileContext(nc) as tc:
        _build_kernel(tc, x_t, z_t, m_t, a_t, rv_t, ones_t, out_t)
    nc.compile()
    _PROGRAM = nc
    return nc


def make_in_maps(x, M, Z, alpha, rv):
    x = np.asarray(x, dtype=np.float32)
    # pre-pad into zero-bordered 30x30(+12) rows: the device signs the
    # whole tile and sign(0)=0 reproduces the conv's zero padding
    xp = np.zeros((B, C, PADW), dtype=np.float32)
    xp[:, :, 31 : 31 + 28 * HP].reshape(B, C, H, HP)[:, :, :, :H] = x
    M = np.asarray(M, dtype=np.float32).reshape(O, CT)
    Z = np.asarray(Z, dtype=np.float32).reshape(K, O, CT)
    alpha = np.asarray(alpha, dtype=np.float32).reshape(O)
    rv = np.ascontiguousarray(np.asarray(rv, dtype=np.float32))
    in_maps = []
    for i in range(N_CORES):
        b, oh = divmod(i, 2)
        osl = slice(oh * O_SH, (oh + 1) * O_SH)
        # Zq[c_low, k, (cc t oo)] = Z[k, 128*oh+oo, (cc*128+c_low)*9 + t]
        zq = np.ascontiguousarray(
            Z[:, osl].reshape(K, O_SH, 2, 128, KK)
            .transpose(3, 0, 2, 4, 1)
            .reshape(128, K, WCOLS)
        )
        mq = np.ascontiguousarray(
            M[osl].reshape(O_SH, 2, 128, KK).transpose(2, 1, 3, 0).reshape(128, WCOLS)
        )
        in_maps.append(
            {
                "x": np.ascontiguousarray(xp[b * B_SH : (b + 1) * B_SH]),
                "Zq": zq,
                "Mq": mq,
                "alphap": np.ascontiguousarray(alpha[osl].reshape(O_SH, 1)),
                "rv": rv,
            }
        )
    return in_maps


def assemble_out(results):
    out = np.empty((B, O, H, H), dtype=np.float32)
    for i in range(N_CORES):
        b, oh = divmod(i, 2)
        r = np.asarray(results[i]["out"]).reshape(B_SH, O_SH, H, H)
        out[b * B_SH : (b + 1) * B_SH, oh * O_SH : (oh + 1) * O_SH] = r
    return out


def kernel(x, M, Z, alpha, rv, trace=False):
    nc = build_program()
    in_maps = make_in_maps(x, M, Z, alpha, rv)
    res = run_bass_kernel_spmd(
        nc, in_maps, core_ids=list(range(N_CORES)), trace=trace
    )
    if trace:
        kernel.last_results = res
    return assemble_out(res.results)


if __name__ == "__main__":
    build_program()
    print("program built ok")
